# revision 36
# baseline (speedup 1.0000x reference)
"""CausalShapedAttention Trainium2 kernel (fp8 DoubleRow, v3).

y = beta * softmax(causal(q k^T / sqrt(hd))) @ v + alpha * v - gamma * MC @ v

where q,k = x @ W_attn^T (packed), v = x (reshaped to heads), MC = causal
uniform attention (row i: 1/(i+1) for j<=i).

Sharding: hybrid batch x head-quad: core c handles batch c//4 and heads
4*(c%4)..4*(c%4)+3.  Each core computes y[b, :, 256*(c%4) : 256*(c%4)+256].

Techniques (all fp8e4m3 operands on the PE):
  - Projection with DoubleRow (K=256/instr): W host-scaled by 64 so
    scores_psum = 32768*s_true; exp applies scale=2^-15.
  - Scores with DoubleRow at K=64 by splitting hd into two 32-halves
    stacked in the free dim: q/k stored [64, 2, T] per head-pair, per-head
    slices at partition bases 0/32.
  - P^T (=exp scores, [j, i] layout) written by ACT as fp8 into jb-pair
    tiles [128, 2, W]; per-i-window U^T accumulation with v-pairs as
    DoubleRow stationaries (few LDWEIGHTS), then PE-transposed back to
    [i, d] via identity matmuls.
  - softmax denominators ride along as a 65th v column (value 1/beta).
  - MC @ v: per-block tri matmul over vL (a copy of v whose row 0
    accumulates the running prefix: row 127 of block jb's psum result is
    exactly prefix[jb+1], added into vL[jb+1] row 0 by one tiny DVE op).
  - fused tails with scalar_tensor_tensor; gpsimd handles the causal
    tri-masks and pair-tile zero-fills.
"""

import os
import sys
import types

sys.path.insert(0, "/opt/trn_rl_repo")

import numpy as np
import ml_dtypes

B, T, C, H, HD = 2, 2048, 1024, 16, 64
NCORES = 8
HPC = 4                      # heads per core
TB = T // 128                # 16 row/col blocks

_PROGRAM = None
LAST_EXEC_NS = None
LAST_TRACE_DIR = None


def _install_patches():
    """Work around environment quirks:
    - walrus here rejects instructions with >1-2 sem waits (see
      _split_excess_waits).
    - antenv.axon_hooks is absent in this image: stub it and register the
      NTFF profile hook from trn_agent_boot so trace=True works.
    """
    try:
        import antenv  # noqa: F401
        if "antenv.axon_hooks" not in sys.modules:
            hooks_mod = types.ModuleType("antenv.axon_hooks")
            _h = [None]
            hooks_mod.set_axon_ntff_profile_hook = lambda h: _h.__setitem__(0, h)
            hooks_mod.get_axon_ntff_profile_hook = lambda: _h[0]
            sys.modules["antenv.axon_hooks"] = hooks_mod
            antenv.axon_hooks = hooks_mod
            from trn_agent_boot.trn_boot import _ntff_profile_via_ctypes
            hooks_mod.set_axon_ntff_profile_hook(
                _ntff_profile_via_ctypes("/opt/axon/libaxon_pjrt.so")
            )
        import concourse.bass_utils as bu
        bu.upload_artifacts = lambda d: d  # no artifact bucket here
    except Exception:
        pass


def _split_excess_waits(nc, limit=1):
    """walrus here rejects instructions with more than ~2 sem waits; split
    excess waits onto same-engine NoOps inserted just before the instruction
    (engine streams are per-engine program order, so semantics are identical).
    """
    import concourse.mybir as mybir

    n = 0
    for bb in nc.main_func.blocks:
        out = []
        for inst in bb.instructions:
            si = inst.sync_info
            if (
                si is not None
                and si.on_wait
                and len(si.on_wait) > limit
                and inst.engine != mybir.EngineType.Unassigned
            ):
                waits = list(si.on_wait)
                for w in waits[:-limit]:
                    n += 1
                    nop = mybir.InstNoOp(
                        name=f"{inst.name}-wsplit{n}",
                        engine=inst.engine,
                        ins=[], outs=[],
                        sync_info=mybir.SyncInfo(on_wait=[w], on_update=[]),
                    )
                    nc.register_instruction(nop)
                    out.append(nop)
                inst.sync_info = mybir.SyncInfo(
                    on_wait=waits[-limit:], on_update=list(si.on_update)
                )
            out.append(inst)
        bb.instructions = out


def _build_program():
    import concourse.bass as bass
    import concourse.mybir as mybir
    import concourse.tile as tile
    from concourse.bass import ts, ds

    f32 = mybir.dt.float32
    fp8 = mybir.dt.float8e4
    Exp = mybir.ActivationFunctionType.Exp
    DR = mybir.MatmulPerfMode.DoubleRow
    mult = mybir.AluOpType.mult
    sub = mybir.AluOpType.subtract

    nc = bass.Bass()
    # DRAM inputs (packed per-core; see _prep_inputs for exact layouts)
    xT8 = nc.dram_tensor("xT8", [128, 8, T], fp8, kind="ExternalInput")
    w8 = nc.dram_tensor("w8", [128, 4, 4, 2, 128], fp8, kind="ExternalInput")
    v16 = nc.dram_tensor("v16", [HPC, 128, TB, 80], fp8, kind="ExternalInput")
    vL8 = nc.dram_tensor("vL8", [HPC, 128, TB, 64], fp8, kind="ExternalInput")
    v32 = nc.dram_tensor("v32", [HPC, 128, TB * 64], mybir.dt.bfloat16,
                         kind="ExternalInput")
    tri_d = nc.dram_tensor("tri", [128, 128], fp8, kind="ExternalInput")
    idn_d = nc.dram_tensor("idn", [128, 128], f32, kind="ExternalInput")
    cinv_d = nc.dram_tensor("cinv", [128, TB], f32, kind="ExternalInput")
    y = nc.dram_tensor("y", [T, HPC * 64], f32, kind="ExternalOutput")

    ESC = float(2.0 ** -15)  # exp scale: undoes host-side W*64 packing

    with tile.TileContext(nc) as tc:
        with (
            tc.tile_pool(name="consts", bufs=1) as consts,
            tc.tile_pool(name="xtp", bufs=1) as xtp,
            tc.tile_pool(name="qk", bufs=1) as qkp,
            tc.tile_pool(name="vp", bufs=1) as vp,
            tc.tile_pool(name="pt", bufs=1) as ptp,
            tc.tile_pool(name="uts", bufs=2) as utsp,
            tc.tile_pool(name="pfx", bufs=8) as pfxp,
            tc.tile_pool(name="small", bufs=4) as small,
            tc.tile_pool(name="tmp", bufs=4) as tmp,
            tc.tile_pool(name="yst", bufs=1) as ystp,
            tc.tile_pool(name="sc_ps", bufs=3, space="PSUM") as sc_ps,
            tc.tile_pool(name="ul_ps", bufs=2, space="PSUM") as ul_ps,
        ):
            dq = [nc.sync, nc.scalar]
            w8_t = consts.tile([128, 4, 4, 2, 128], fp8, tag="w8")
            nc.sync.dma_start(w8_t[:], w8[:])
            tri_t = consts.tile([128, 128], fp8, tag="tri")
            nc.sync.dma_start(tri_t[:], tri_d[:])

            xp = []
            vL_t = []
            for n in range(4):
                t = xtp.tile([128, 8, 512], fp8, tag=f"x{n}", name=f"x{n}")
                for g in range(4):
                    dq[g % 2].dma_start(t[:, ds(2 * g, 2)],
                                        xT8[:, ds(2 * g, 2), ts(n, 512)])
                xp.append(t)
                if n == 0:
                    for h in range(HPC):
                        t2 = vp.tile([128, TB, 64], fp8, tag=f"vL_{h}",
                                     name=f"vL_{h}")
                        nc.scalar.dma_start(t2[:], vL8[h])
                        vL_t.append(t2)

            idn_t = consts.tile([128, 128], f32, tag="idn")
            nc.sync.dma_start(idn_t[:], idn_d[:])
            cinv_t = consts.tile([128, TB], f32, tag="cinv")
            nc.sync.dma_start(cinv_t[:], cinv_d[:])

            v16_t, v32_t = [], []
            for h in range(HPC):
                t = vp.tile([128, TB, 80], fp8, tag=f"v16_{h}", name=f"v16_{h}")
                dq[h % 2].dma_start(t[:], v16[h])
                v16_t.append(t)
                t = vp.tile([128, TB * 64], mybir.dt.bfloat16,
                            tag=f"v32_{h}", name=f"v32_{h}")
                dq[(h + 1) % 2].dma_start(t[:], v32[h])
                v32_t.append(t)

            # ------------- projection (fp8 DoubleRow, K=1024) -------------
            # psum tensors m: 0=kA(d-lo) 1=kB(d-hi) 2=qA 3=qB; psum partition
            # p -> head p//32, d = 32*(m-half) + p%32.  sbuf: [64, 2, T] per
            # head pair, partitions [h_even|h_odd], dim1 = d-half.
            k01 = qkp.tile([128, T], fp8, tag="k01", name="k01")
            k23 = qkp.tile([128, T], fp8, tag="k23", name="k23")
            q01 = qkp.tile([128, T], fp8, tag="q01", name="q01")
            q23 = qkp.tile([128, T], fp8, tag="q23", name="q23")
            dest = {0: k01, 1: k23, 2: q01, 3: q23}
            # n-outer so each t-quarter's 4 psum groups start as soon as its
            # x slice lands; k01/q01 upfront (head 0/1 inputs), k23/q23
            # deferred into head 0's step stream as PE filler.
            def proj_group(m, n):
                ps = sc_ps.tile([128, 1024], f32, tag="sp", name=f"pj{m}{n}")
                for g in range(4):
                    nc.tensor.matmul(
                        ps[:, ds(0, 512)], w8_t[:, m, g],
                        xp[n][:, ds(2 * g, 2)],
                        start=(g == 0), stop=(g == 3), perf_mode=DR,
                    )
                dst_t = dest[m]
                if m < 2:
                    nc.scalar.copy(dst_t[:, ts(n, 512)], ps[:, ds(0, 512)])
                else:
                    nc.vector.tensor_copy(dst_t[:, ts(n, 512)],
                                          ps[:, ds(0, 512)])

            # colsums slot in after the first projection quarter: by then
            # vL has landed, and they fill the wait for the next x quarter
            css_store = {h: [] for h in range(HPC)}
            for n in range(4):
                for m in (0, 2, 1, 3):
                    proj_group(m, n)
                if n == 0:
                    for h in range(HPC):
                        for g8 in range(2):
                            cp = sc_ps.tile([1, 512], f32, tag="sp",
                                            name=f"cs{h}{g8}")
                            nc.tensor.matmul(cp[:], tri_t[:, ds(127, 1)],
                                             vL_t[h][:, ds(8 * g8, 8)],
                                             start=True, stop=True)
                            cs_sb = pfxp.tile([1, 512], f32, tag="cs_sb",
                                              name=f"cssb{h}{g8}")
                            nc.vector.tensor_copy(cs_sb[:], cp[:])
                            css_store[h].append(cs_sb)
            deferred = []

            yst = [ystp.tile([128, HPC * 64], f32, tag=f"yst{ib}",
                             name=f"yst{ib}") for ib in range(TB)]

            # ------------------- attention, flat pipeline ------------------
            # One global step stream over (h, jb).  U^T bursts are emitted
            # right after their last exp; transposes+tails lag two steps so
            # the DVE psum->sbuf copy never stalls the PE.  Head h+1's steps
            # interleave with head h's final bursts.
            state = {}
            for h in range(HPC):
                state[h] = {
                    "ptt": [ptp.tile([128, 2, T - 256 * g], fp8,
                                     tag=f"pt{h % 2}_{g}", name=f"pt{h}_{g}")
                            for g in range(8)],
                    "ul": {},
                    "css": css_store[h],
                }


            def prefix_chain(h):
                st = state[h]
                prev = None
                for ib in range(1, TB):
                    sl = st["css"][(ib - 1) // 8][
                        0:1, ds(((ib - 1) % 8) * 64, 64)]
                    a = pfxp.tile([1, 64], f32, tag="acc",
                                  name=f"acc{h}{ib}")
                    if prev is None:
                        nc.gpsimd.tensor_copy(a[:], sl)
                    else:
                        nc.gpsimd.tensor_add(a[:], prev[:], sl)
                    prev = a
                    nc.gpsimd.tensor_add(
                        vL_t[h][ds(0, 1), ib],
                        a[:], vL_t[h][ds(0, 1), ib],
                    )

            def u_bulk(h, iw):
                # U^T[iw] partial: fully-causal pairs (exps long complete)
                ptt = state[h]["ptt"]
                up = ul_ps.tile([65, 512], f32, tag="ul", name=f"ut{h}{iw}")
                for g in range(2 * iw):
                    nc.tensor.matmul(
                        up[:, ds(0, 512)],
                        v16_t[h][:, ds(2 * g, 2), ds(0, 65)],
                        ptt[g][:, :, ds(512 * iw - 256 * g, 512)],
                        start=(g == 0), stop=False,
                        perf_mode=DR,
                    )
                return up

            def u_diag(h, iw, up):
                # U^T[iw] diagonal pairs + psum -> sbuf copy
                ptt = state[h]["ptt"]
                for g in (2 * iw, 2 * iw + 1):
                    if g == 2 * iw + 1:
                        dst, src, nn = 256, 0, 256
                    else:
                        dst, src, nn = 0, 0, 512
                    nc.tensor.matmul(
                        up[:, ds(dst, nn)],
                        v16_t[h][:, ds(2 * g, 2), ds(0, 65)],
                        ptt[g][:, :, ds(src, nn)],
                        start=(iw == 0 and g == 0),
                        stop=(g == 2 * iw + 1),
                        perf_mode=DR,
                    )
                uts = utsp.tile([65, 512], f32, tag="uts",
                                name=f"uts{h}{iw}")
                nc.vector.tensor_copy(uts[:], up[:])
                return uts

            def t_burst(h, iw, uts):
                # transposes + Lv matmuls + fused tails for one i-window
                ul = state[h]["ul"]
                for k2 in (2 * iw, 2 * iw + 1):
                    ul[k2] = ul_ps.tile([128, 2, 129], f32, tag="ul",
                                        name=f"ul{h}{k2}")
                for c in range(4):
                    ib = 4 * iw + c
                    nc.tensor.transpose(
                        ul[ib // 2][:, ib % 2, ds(0, 65)],
                        uts[:, ts(c, 128)], idn_t[ds(0, 65), ds(0, 65)],
                    )
                    nc.tensor.matmul(ul[ib // 2][:, ib % 2, ds(65, 64)],
                                     tri_t[:], vL_t[h][:, ib],
                                     start=True, stop=True)
                for k2 in (2 * iw, 2 * iw + 1):
                    ult = ul[k2]
                    r2 = small.tile([128, 2], f32, tag="r2",
                                    name=f"r2_{h}_{k2}")
                    nc.vector.reciprocal(r2[:], ult[:, :, ds(64, 1)])
                    for mm in range(2):
                        ib = 2 * k2 + mm
                        m1 = tmp.tile([128, 64], f32, tag="m1",
                                      name=f"m1_{h}_{ib}")
                        nc.vector.scalar_tensor_tensor(
                            m1[:], ult[:, mm, ds(65, 64)],
                            cinv_t[:, ds(ib, 1)],
                            v32_t[h][:, ds(64 * ib, 64)], mult, sub,
                        )
                        nc.vector.scalar_tensor_tensor(
                            yst[ib][:, ds(64 * h, 64)],
                            ult[:, mm, ds(0, 64)], r2[:, ds(mm, 1)],
                            m1[:], mult, sub,
                        )
                        if h == HPC - 1:
                            nc.sync.dma_start(y[ts(ib, 128), :], yst[ib][:])
                    del ul[k2]

            pend = {}   # due_step -> (kind, args)
            uts_live = {}
            nsteps = HPC * TB
            for step in range(nsteps + 5):
                h, jb = divmod(step, TB)
                due = pend.pop(step, None)
                if due:
                    if due[0] == "u":
                        uts_live[(due[1], due[2])] = u_burst(due[1], due[2])
                    elif due[0] == "t":
                        t_burst(due[1], due[2],
                                uts_live.pop((due[1], due[2])))
                if step >= nsteps:
                    continue

                kt = k01 if h < 2 else k23
                qt = q01 if h < 2 else q23
                p0 = 64 * (h % 2)
                st = state[h]
                ptt = st["ptt"]
                ul = st["ul"]
                g, m = jb // 2, jb % 2

                # scores S^T[j in jb, i>=128*jb], plain fp8 K=64
                for w2 in range(jb // 8, 2):
                    dcol = max(0, 128 * jb - 1024 * w2)
                    nw = 1024 - dcol
                    sp = sc_ps.tile([128, 1024], f32, tag="sp",
                                    name=f"sc{h}{jb}{w2}")
                    segs = ([(dcol, 512 - dcol), (512, 512)]
                            if dcol < 512 else [(dcol, 1024 - dcol)])
                    for (c0, nseg) in segs:
                        nc.tensor.matmul(
                            sp[:, ds(c0, nseg)],
                            kt[ds(p0, 64), ts(jb, 128)],
                            qt[ds(p0, 64), ds(1024 * w2 + c0, nseg)],
                            start=True, stop=True,
                        )
                    off = 1024 * w2 + dcol - 256 * g
                    nc.scalar.activation(
                        ptt[g][:, m, ds(off, nw)], sp[:, ds(dcol, nw)],
                        Exp, scale=ESC,
                    )
                dslc = ptt[g][:, m, ds(128 * m, 128)]
                nc.gpsimd.tensor_mul(dslc, dslc, tri_t[:])
                if m == 1:
                    nc.gpsimd.memset(ptt[g][:, 1, ds(0, 128)], 0.0)

                if jb == 1:
                    prefix_chain(h)

                # U^T burst 2 steps after its last exp; tails 2 more later
                if jb % 4 == 3:
                    iw = jb // 4
                    pend[step + 2] = ("u", h, iw)
                    pend[step + 4] = ("t", h, iw, None)

    _split_excess_waits(nc)
    nc.finalize()
    return nc


def _prep_inputs(x, W_attn, alpha, beta, gamma):
    """Host-side sharding/layout prep. Returns per-core input maps."""
    fp8 = ml_dtypes.float8_e4m3fn
    x = np.asarray(x, dtype=np.float32)
    W_attn = np.asarray(W_attn, dtype=np.float32)
    alpha = float(alpha)
    beta = float(beta)
    gamma = float(gamma)

    tri = np.triu(np.ones((128, 128), dtype=np.float32)).astype(fp8)  # j<=i
    idn = np.eye(128, dtype=np.float32)
    cinv = gamma / (np.arange(1, T + 1, dtype=np.float32)
                    .reshape(TB, 128).T.copy())  # [p, ib]
    inv_beta = np.float32(1.0 / beta) if beta != 0 else np.float32(np.inf)

    in_maps = []
    for core in range(NCORES):
        b = core // 4
        h0 = HPC * (core % 4)
        # xT8[p, c, t] = x[b, t, 128c+p]
        xT8 = np.ascontiguousarray(
            x[b].T.reshape(8, 128, T).transpose(1, 0, 2)).astype(fp8)
        # w8[p, m, g, i, o]: m in (k01, k23, q01, q23); psum partition o of
        # tensor m = contiguous W rows; contraction (2g+i)*128 + p; scaled 64.
        w8 = np.empty((128, 4, 4, 2, 128), dtype=np.float32)
        starts = [C + h0 * 64, C + (h0 + 2) * 64, h0 * 64, (h0 + 2) * 64]
        for m in range(4):
            wm = W_attn[starts[m]:starts[m] + 128, :] * 64.0
            w8[:, m] = wm.T.reshape(4, 2, 128, 128).transpose(2, 0, 1, 3)
        w8 = np.ascontiguousarray(w8).astype(fp8)

        v16 = np.zeros((HPC, 128, TB, 80), dtype=np.float32)
        v32 = np.empty((HPC, 128, TB, 64), dtype=np.float32)
        for h in range(HPC):
            hh = h0 + h
            vb = x[b][:, hh * 64:(hh + 1) * 64].reshape(TB, 128, 64)
            v16[h, :, :, :64] = vb.transpose(1, 0, 2)
            v16[h, :, :, 64] = inv_beta
            v32[h] = alpha * vb.transpose(1, 0, 2)
        vL = v16[:, :, :, :64].copy()
        v16 = np.ascontiguousarray(v16).astype(fp8)
        vL = np.ascontiguousarray(vL).astype(fp8)
        v32 = np.ascontiguousarray(
            v32.reshape(HPC, 128, TB * 64)).astype(ml_dtypes.bfloat16)

        in_maps.append({
            "xT8": xT8,
            "w8": w8,
            "v16": v16,
            "vL8": vL,
            "v32": v32,
            "tri": tri,
            "idn": idn,
            "cinv": cinv.astype(np.float32),
        })
    return in_maps


def kernel(x, W_attn, alpha, beta, gamma):
    global _PROGRAM, LAST_EXEC_NS, LAST_TRACE_DIR
    _install_patches()
    from concourse.bass_utils import run_bass_kernel_spmd

    if _PROGRAM is None:
        _PROGRAM = _build_program()
    nc = _PROGRAM

    in_maps = _prep_inputs(x, W_attn, alpha, beta, gamma)

    trace = os.environ.get("KERNEL_TRACE", "0") == "1"
    kwargs = {}
    if trace:
        trace_dir = os.environ.get("KERNEL_TRACE_DIR") or None
        if trace_dir:
            os.makedirs(trace_dir, exist_ok=True)
            kwargs["tmpdir"] = trace_dir
    res = run_bass_kernel_spmd(
        nc, in_maps, core_ids=list(range(NCORES)), trace=trace, **kwargs
    )
    LAST_EXEC_NS = res.exec_time_ns
    if trace and "tmpdir" in kwargs:
        LAST_TRACE_DIR = kwargs["tmpdir"]

    out = np.empty((B, T, C), dtype=np.float32)
    for core in range(NCORES):
        b = core // 4
        c0 = 256 * (core % 4)
        out[b, :, c0:c0 + 256] = res.results[core]["y"]
    return out


# revision 37
# speedup vs baseline: 1.0630x; 1.0630x over previous
"""CausalShapedAttention Trainium2 kernel (fp8 DoubleRow, v3).

y = beta * softmax(causal(q k^T / sqrt(hd))) @ v + alpha * v - gamma * MC @ v

where q,k = x @ W_attn^T (packed), v = x (reshaped to heads), MC = causal
uniform attention (row i: 1/(i+1) for j<=i).

Sharding: hybrid batch x head-quad: core c handles batch c//4 and heads
4*(c%4)..4*(c%4)+3.  Each core computes y[b, :, 256*(c%4) : 256*(c%4)+256].

Techniques (all fp8e4m3 operands on the PE):
  - Projection with DoubleRow (K=256/instr): W host-scaled by 64 so
    scores_psum = 32768*s_true; exp applies scale=2^-15.
  - Scores with DoubleRow at K=64 by splitting hd into two 32-halves
    stacked in the free dim: q/k stored [64, 2, T] per head-pair, per-head
    slices at partition bases 0/32.
  - P^T (=exp scores, [j, i] layout) written by ACT as fp8 into jb-pair
    tiles [128, 2, W]; per-i-window U^T accumulation with v-pairs as
    DoubleRow stationaries (few LDWEIGHTS), then PE-transposed back to
    [i, d] via identity matmuls.
  - softmax denominators ride along as a 65th v column (value 1/beta).
  - MC @ v: per-block tri matmul over vL (a copy of v whose row 0
    accumulates the running prefix: row 127 of block jb's psum result is
    exactly prefix[jb+1], added into vL[jb+1] row 0 by one tiny DVE op).
  - fused tails with scalar_tensor_tensor; gpsimd handles the causal
    tri-masks and pair-tile zero-fills.
"""

import os
import sys
import types

sys.path.insert(0, "/opt/trn_rl_repo")

import numpy as np
import ml_dtypes

B, T, C, H, HD = 2, 2048, 1024, 16, 64
NCORES = 8
HPC = 4                      # heads per core
TB = T // 128                # 16 row/col blocks

_PROGRAM = None
LAST_EXEC_NS = None
LAST_TRACE_DIR = None


def _install_patches():
    """Work around environment quirks:
    - walrus here rejects instructions with >1-2 sem waits (see
      _split_excess_waits).
    - antenv.axon_hooks is absent in this image: stub it and register the
      NTFF profile hook from trn_agent_boot so trace=True works.
    """
    try:
        import antenv  # noqa: F401
        if "antenv.axon_hooks" not in sys.modules:
            hooks_mod = types.ModuleType("antenv.axon_hooks")
            _h = [None]
            hooks_mod.set_axon_ntff_profile_hook = lambda h: _h.__setitem__(0, h)
            hooks_mod.get_axon_ntff_profile_hook = lambda: _h[0]
            sys.modules["antenv.axon_hooks"] = hooks_mod
            antenv.axon_hooks = hooks_mod
            from trn_agent_boot.trn_boot import _ntff_profile_via_ctypes
            hooks_mod.set_axon_ntff_profile_hook(
                _ntff_profile_via_ctypes("/opt/axon/libaxon_pjrt.so")
            )
        import concourse.bass_utils as bu
        bu.upload_artifacts = lambda d: d  # no artifact bucket here
    except Exception:
        pass


def _split_excess_waits(nc, limit=1):
    """walrus here rejects instructions with more than ~2 sem waits; split
    excess waits onto same-engine NoOps inserted just before the instruction
    (engine streams are per-engine program order, so semantics are identical).
    """
    import concourse.mybir as mybir

    n = 0
    for bb in nc.main_func.blocks:
        out = []
        for inst in bb.instructions:
            si = inst.sync_info
            if (
                si is not None
                and si.on_wait
                and len(si.on_wait) > limit
                and inst.engine != mybir.EngineType.Unassigned
            ):
                waits = list(si.on_wait)
                for w in waits[:-limit]:
                    n += 1
                    nop = mybir.InstNoOp(
                        name=f"{inst.name}-wsplit{n}",
                        engine=inst.engine,
                        ins=[], outs=[],
                        sync_info=mybir.SyncInfo(on_wait=[w], on_update=[]),
                    )
                    nc.register_instruction(nop)
                    out.append(nop)
                inst.sync_info = mybir.SyncInfo(
                    on_wait=waits[-limit:], on_update=list(si.on_update)
                )
            out.append(inst)
        bb.instructions = out


def _build_program():
    import concourse.bass as bass
    import concourse.mybir as mybir
    import concourse.tile as tile
    from concourse.bass import ts, ds

    f32 = mybir.dt.float32
    fp8 = mybir.dt.float8e4
    Exp = mybir.ActivationFunctionType.Exp
    DR = mybir.MatmulPerfMode.DoubleRow
    mult = mybir.AluOpType.mult
    sub = mybir.AluOpType.subtract

    nc = bass.Bass()
    # DRAM inputs (packed per-core; see _prep_inputs for exact layouts)
    xT8 = nc.dram_tensor("xT8", [128, 8, T], fp8, kind="ExternalInput")
    w8 = nc.dram_tensor("w8", [128, 4, 4, 2, 128], fp8, kind="ExternalInput")
    v16 = nc.dram_tensor("v16", [HPC, 128, TB, 80], fp8, kind="ExternalInput")
    vL8 = nc.dram_tensor("vL8", [HPC, 128, TB, 64], fp8, kind="ExternalInput")
    v32 = nc.dram_tensor("v32", [HPC, 128, TB * 64], mybir.dt.bfloat16,
                         kind="ExternalInput")
    tri_d = nc.dram_tensor("tri", [128, 128], fp8, kind="ExternalInput")
    idn_d = nc.dram_tensor("idn", [128, 128], f32, kind="ExternalInput")
    cinv_d = nc.dram_tensor("cinv", [128, TB], f32, kind="ExternalInput")
    y = nc.dram_tensor("y", [T, HPC * 64], f32, kind="ExternalOutput")

    ESC = float(2.0 ** -15)  # exp scale: undoes host-side W*64 packing

    with tile.TileContext(nc) as tc:
        with (
            tc.tile_pool(name="consts", bufs=1) as consts,
            tc.tile_pool(name="xtp", bufs=1) as xtp,
            tc.tile_pool(name="qk", bufs=1) as qkp,
            tc.tile_pool(name="vp", bufs=1) as vp,
            tc.tile_pool(name="pt", bufs=1) as ptp,
            tc.tile_pool(name="uts", bufs=2) as utsp,
            tc.tile_pool(name="pfx", bufs=8) as pfxp,
            tc.tile_pool(name="small", bufs=4) as small,
            tc.tile_pool(name="tmp", bufs=4) as tmp,
            tc.tile_pool(name="yst", bufs=1) as ystp,
            tc.tile_pool(name="sc_ps", bufs=3, space="PSUM") as sc_ps,
            tc.tile_pool(name="ul_ps", bufs=2, space="PSUM") as ul_ps,
        ):
            w8_t = consts.tile([128, 4, 4, 2, 128], fp8, tag="w8")
            tri_t = consts.tile([128, 128], fp8, tag="tri")
            nc.sync.dma_start(tri_t[:], tri_d[:])
            idn_t = consts.tile([128, 128], f32, tag="idn")
            nc.sync.dma_start(idn_t[:], idn_d[:])
            cinv_t = consts.tile([128, TB], f32, tag="cinv")
            nc.sync.dma_start(cinv_t[:], cinv_d[:])

            dq = [nc.sync, nc.scalar]
            vL_t = []
            for h in range(HPC):
                t = vp.tile([128, TB, 64], fp8, tag=f"vL_{h}", name=f"vL_{h}")
                dq[h % 2].dma_start(t[:], vL8[h])
                vL_t.append(t)

            xp = []
            for n in range(4):
                t = xtp.tile([128, 8, 512], fp8, tag=f"x{n}", name=f"x{n}")
                for g in range(4):
                    dq[g % 2].dma_start(t[:, ds(2 * g, 2)],
                                        xT8[:, ds(2 * g, 2), ts(n, 512)])
                xp.append(t)
                if n == 0:
                    nc.sync.dma_start(w8_t[:], w8[:])

            v16_t, v32_t = [], []
            for h in range(HPC):
                t = vp.tile([128, TB, 80], fp8, tag=f"v16_{h}", name=f"v16_{h}")
                dq[h % 2].dma_start(t[:], v16[h])
                v16_t.append(t)
                t = vp.tile([128, TB * 64], mybir.dt.bfloat16,
                            tag=f"v32_{h}", name=f"v32_{h}")
                dq[(h + 1) % 2].dma_start(t[:], v32[h])
                v32_t.append(t)

            # ------------- projection (fp8 DoubleRow, K=1024) -------------
            # psum tensors m: 0=kA(d-lo) 1=kB(d-hi) 2=qA 3=qB; psum partition
            # p -> head p//32, d = 32*(m-half) + p%32.  sbuf: [64, 2, T] per
            # head pair, partitions [h_even|h_odd], dim1 = d-half.
            k01 = qkp.tile([128, T], fp8, tag="k01", name="k01")
            k23 = qkp.tile([128, T], fp8, tag="k23", name="k23")
            q01 = qkp.tile([128, T], fp8, tag="q01", name="q01")
            q23 = qkp.tile([128, T], fp8, tag="q23", name="q23")
            dest = {0: k01, 1: k23, 2: q01, 3: q23}
            # n-outer so each t-quarter's 4 psum groups start as soon as its
            # x slice lands; k01/q01 upfront (head 0/1 inputs), k23/q23
            # deferred into head 0's step stream as PE filler.
            def proj_group(m, n):
                ps = sc_ps.tile([128, 1024], f32, tag="sp", name=f"pj{m}{n}")
                for g in range(4):
                    nc.tensor.matmul(
                        ps[:, ds(0, 512)], w8_t[:, m, g],
                        xp[n][:, ds(2 * g, 2)],
                        start=(g == 0), stop=(g == 3), perf_mode=DR,
                    )
                dst_t = dest[m]
                if m < 2:
                    nc.scalar.copy(dst_t[:, ts(n, 512)], ps[:, ds(0, 512)])
                else:
                    nc.vector.tensor_copy(dst_t[:, ts(n, 512)],
                                          ps[:, ds(0, 512)])

            # block colsums upfront: x-independent PE work that fills the
            # input-DMA window (chains stay at each head's jb==1 slot)
            css_store = {h: [] for h in range(HPC)}
            for h in range(HPC):
                for g8 in range(2):
                    cp = sc_ps.tile([1, 512], f32, tag="sp",
                                    name=f"cs{h}{g8}")
                    nc.tensor.matmul(cp[:], tri_t[:, ds(127, 1)],
                                     vL_t[h][:, ds(8 * g8, 8)],
                                     start=True, stop=True)
                    cs_sb = pfxp.tile([1, 512], f32, tag="cs_sb",
                                      name=f"cssb{h}{g8}")
                    nc.vector.tensor_copy(cs_sb[:], cp[:])
                    css_store[h].append(cs_sb)

            for n in range(4):
                for m in (0, 2, 1, 3):
                    proj_group(m, n)
            deferred = []

            yst = [ystp.tile([128, HPC * 64], f32, tag=f"yst{ib}",
                             name=f"yst{ib}") for ib in range(TB)]

            # ------------------- attention, flat pipeline ------------------
            # One global step stream over (h, jb).  U^T bursts are emitted
            # right after their last exp; transposes+tails lag two steps so
            # the DVE psum->sbuf copy never stalls the PE.  Head h+1's steps
            # interleave with head h's final bursts.
            state = {}
            for h in range(HPC):
                state[h] = {
                    "ptt": [ptp.tile([128, 2, T - 256 * g], fp8,
                                     tag=f"pt{h % 2}_{g}", name=f"pt{h}_{g}")
                            for g in range(8)],
                    "ul": {},
                    "css": css_store[h],
                }


            def prefix_chain(h):
                st = state[h]
                prev = None
                for ib in range(1, TB):
                    sl = st["css"][(ib - 1) // 8][
                        0:1, ds(((ib - 1) % 8) * 64, 64)]
                    a = pfxp.tile([1, 64], f32, tag="acc",
                                  name=f"acc{h}{ib}")
                    if prev is None:
                        nc.gpsimd.tensor_copy(a[:], sl)
                    else:
                        nc.gpsimd.tensor_add(a[:], prev[:], sl)
                    prev = a
                    nc.gpsimd.tensor_add(
                        vL_t[h][ds(0, 1), ib],
                        a[:], vL_t[h][ds(0, 1), ib],
                    )

            def u_bulk(h, iw):
                # U^T[iw] partial: fully-causal pairs (exps long complete)
                ptt = state[h]["ptt"]
                up = ul_ps.tile([65, 512], f32, tag="ul", name=f"ut{h}{iw}")
                for g in range(2 * iw):
                    nc.tensor.matmul(
                        up[:, ds(0, 512)],
                        v16_t[h][:, ds(2 * g, 2), ds(0, 65)],
                        ptt[g][:, :, ds(512 * iw - 256 * g, 512)],
                        start=(g == 0), stop=False,
                        perf_mode=DR,
                    )
                return up

            def u_diag(h, iw, up):
                # U^T[iw] diagonal pairs + psum -> sbuf copy
                ptt = state[h]["ptt"]
                for g in (2 * iw, 2 * iw + 1):
                    if g == 2 * iw + 1:
                        dst, src, nn = 256, 0, 256
                    else:
                        dst, src, nn = 0, 0, 512
                    nc.tensor.matmul(
                        up[:, ds(dst, nn)],
                        v16_t[h][:, ds(2 * g, 2), ds(0, 65)],
                        ptt[g][:, :, ds(src, nn)],
                        start=(iw == 0 and g == 0),
                        stop=(g == 2 * iw + 1),
                        perf_mode=DR,
                    )
                uts = utsp.tile([65, 512], f32, tag="uts",
                                name=f"uts{h}{iw}")
                nc.vector.tensor_copy(uts[:], up[:])
                return uts

            def t_burst(h, iw, uts):
                # transposes + Lv matmuls + fused tails for one i-window
                ul = state[h]["ul"]
                for k2 in (2 * iw, 2 * iw + 1):
                    ul[k2] = ul_ps.tile([128, 2, 129], f32, tag="ul",
                                        name=f"ul{h}{k2}")
                for c in range(4):
                    ib = 4 * iw + c
                    nc.tensor.transpose(
                        ul[ib // 2][:, ib % 2, ds(0, 65)],
                        uts[:, ts(c, 128)], idn_t[ds(0, 65), ds(0, 65)],
                    )
                    nc.tensor.matmul(ul[ib // 2][:, ib % 2, ds(65, 64)],
                                     tri_t[:], vL_t[h][:, ib],
                                     start=True, stop=True)
                for k2 in (2 * iw, 2 * iw + 1):
                    ult = ul[k2]
                    r2 = small.tile([128, 2], f32, tag="r2",
                                    name=f"r2_{h}_{k2}")
                    nc.vector.reciprocal(r2[:], ult[:, :, ds(64, 1)])
                    for mm in range(2):
                        ib = 2 * k2 + mm
                        m1 = tmp.tile([128, 64], f32, tag="m1",
                                      name=f"m1_{h}_{ib}")
                        nc.vector.scalar_tensor_tensor(
                            m1[:], ult[:, mm, ds(65, 64)],
                            cinv_t[:, ds(ib, 1)],
                            v32_t[h][:, ds(64 * ib, 64)], mult, sub,
                        )
                        nc.vector.scalar_tensor_tensor(
                            yst[ib][:, ds(64 * h, 64)],
                            ult[:, mm, ds(0, 64)], r2[:, ds(mm, 1)],
                            m1[:], mult, sub,
                        )
                        if h == HPC - 1:
                            nc.sync.dma_start(y[ts(ib, 128), :], yst[ib][:])
                    del ul[k2]

            pend = {}   # due_step -> (kind, args)
            uts_live = {}
            nsteps = HPC * TB
            for step in range(nsteps + 5):
                h, jb = divmod(step, TB)
                due = pend.pop(step, None)
                if due:
                    if due[0] == "u":
                        uts_live[(due[1], due[2])] = u_burst(due[1], due[2])
                    elif due[0] == "t":
                        t_burst(due[1], due[2],
                                uts_live.pop((due[1], due[2])))
                if step >= nsteps:
                    continue

                kt = k01 if h < 2 else k23
                qt = q01 if h < 2 else q23
                p0 = 64 * (h % 2)
                st = state[h]
                ptt = st["ptt"]
                ul = st["ul"]
                g, m = jb // 2, jb % 2

                # scores S^T[j in jb, i>=128*jb], plain fp8 K=64
                for w2 in range(jb // 8, 2):
                    dcol = max(0, 128 * jb - 1024 * w2)
                    nw = 1024 - dcol
                    sp = sc_ps.tile([128, 1024], f32, tag="sp",
                                    name=f"sc{h}{jb}{w2}")
                    segs = ([(dcol, 512 - dcol), (512, 512)]
                            if dcol < 512 else [(dcol, 1024 - dcol)])
                    for (c0, nseg) in segs:
                        nc.tensor.matmul(
                            sp[:, ds(c0, nseg)],
                            kt[ds(p0, 64), ts(jb, 128)],
                            qt[ds(p0, 64), ds(1024 * w2 + c0, nseg)],
                            start=True, stop=True,
                        )
                    off = 1024 * w2 + dcol - 256 * g
                    nc.scalar.activation(
                        ptt[g][:, m, ds(off, nw)], sp[:, ds(dcol, nw)],
                        Exp, scale=ESC,
                    )
                dslc = ptt[g][:, m, ds(128 * m, 128)]
                nc.gpsimd.tensor_mul(dslc, dslc, tri_t[:])
                if m == 1:
                    nc.gpsimd.memset(ptt[g][:, 1, ds(0, 128)], 0.0)

                if jb == 1:
                    prefix_chain(h)

                # U^T burst 2 steps after its last exp; tails 2 more later
                if jb % 4 == 3:
                    iw = jb // 4
                    pend[step + 2] = ("u", h, iw)
                    pend[step + 4] = ("t", h, iw, None)

    _split_excess_waits(nc)
    nc.finalize()
    return nc


def _prep_inputs(x, W_attn, alpha, beta, gamma):
    """Host-side sharding/layout prep. Returns per-core input maps."""
    fp8 = ml_dtypes.float8_e4m3fn
    x = np.asarray(x, dtype=np.float32)
    W_attn = np.asarray(W_attn, dtype=np.float32)
    alpha = float(alpha)
    beta = float(beta)
    gamma = float(gamma)

    tri = np.triu(np.ones((128, 128), dtype=np.float32)).astype(fp8)  # j<=i
    idn = np.eye(128, dtype=np.float32)
    cinv = gamma / (np.arange(1, T + 1, dtype=np.float32)
                    .reshape(TB, 128).T.copy())  # [p, ib]
    inv_beta = np.float32(1.0 / beta) if beta != 0 else np.float32(np.inf)

    in_maps = []
    for core in range(NCORES):
        b = core // 4
        h0 = HPC * (core % 4)
        # xT8[p, c, t] = x[b, t, 128c+p]
        xT8 = np.ascontiguousarray(
            x[b].T.reshape(8, 128, T).transpose(1, 0, 2)).astype(fp8)
        # w8[p, m, g, i, o]: m in (k01, k23, q01, q23); psum partition o of
        # tensor m = contiguous W rows; contraction (2g+i)*128 + p; scaled 64.
        w8 = np.empty((128, 4, 4, 2, 128), dtype=np.float32)
        starts = [C + h0 * 64, C + (h0 + 2) * 64, h0 * 64, (h0 + 2) * 64]
        for m in range(4):
            wm = W_attn[starts[m]:starts[m] + 128, :] * 64.0
            w8[:, m] = wm.T.reshape(4, 2, 128, 128).transpose(2, 0, 1, 3)
        w8 = np.ascontiguousarray(w8).astype(fp8)

        v16 = np.zeros((HPC, 128, TB, 80), dtype=np.float32)
        v32 = np.empty((HPC, 128, TB, 64), dtype=np.float32)
        for h in range(HPC):
            hh = h0 + h
            vb = x[b][:, hh * 64:(hh + 1) * 64].reshape(TB, 128, 64)
            v16[h, :, :, :64] = vb.transpose(1, 0, 2)
            v16[h, :, :, 64] = inv_beta
            v32[h] = alpha * vb.transpose(1, 0, 2)
        vL = v16[:, :, :, :64].copy()
        v16 = np.ascontiguousarray(v16).astype(fp8)
        vL = np.ascontiguousarray(vL).astype(fp8)
        v32 = np.ascontiguousarray(
            v32.reshape(HPC, 128, TB * 64)).astype(ml_dtypes.bfloat16)

        in_maps.append({
            "xT8": xT8,
            "w8": w8,
            "v16": v16,
            "vL8": vL,
            "v32": v32,
            "tri": tri,
            "idn": idn,
            "cinv": cinv.astype(np.float32),
        })
    return in_maps


def kernel(x, W_attn, alpha, beta, gamma):
    global _PROGRAM, LAST_EXEC_NS, LAST_TRACE_DIR
    _install_patches()
    from concourse.bass_utils import run_bass_kernel_spmd

    if _PROGRAM is None:
        _PROGRAM = _build_program()
    nc = _PROGRAM

    in_maps = _prep_inputs(x, W_attn, alpha, beta, gamma)

    trace = os.environ.get("KERNEL_TRACE", "0") == "1"
    kwargs = {}
    if trace:
        trace_dir = os.environ.get("KERNEL_TRACE_DIR") or None
        if trace_dir:
            os.makedirs(trace_dir, exist_ok=True)
            kwargs["tmpdir"] = trace_dir
    res = run_bass_kernel_spmd(
        nc, in_maps, core_ids=list(range(NCORES)), trace=trace, **kwargs
    )
    LAST_EXEC_NS = res.exec_time_ns
    if trace and "tmpdir" in kwargs:
        LAST_TRACE_DIR = kwargs["tmpdir"]

    out = np.empty((B, T, C), dtype=np.float32)
    for core in range(NCORES):
        b = core // 4
        c0 = 256 * (core % 4)
        out[b, :, c0:c0 + 256] = res.results[core]["y"]
    return out
